# revision 17
# baseline (speedup 1.0000x reference)
"""Trainium2 Bass kernel for EnhancedMultiHeadAttention.

Data-parallel over batch: B=256 split as 32 batches per core across 8 cores.
Each core runs an identical fused kernel over its 2048 tokens:

  - q/k/v + global q/k/v projections as fp16 matmuls (fp32 PSUM accumulate),
    with the 1/sqrt(hd) score scales folded into the k-projection weights.
  - local attention with relative-position bias: the Toeplitz term
    scores[l,r] += q[l] . rel_k[clip(r-l)] is computed as T = q @ REL_EXT.T
    followed by a skewed (diagonal) DMA read through a DRAM scratch buffer.
  - softmax without max-subtraction (scores are bounded), exp row-sums via
    the activation accumulator, per-partition scalar normalization.
  - the two output projections are folded into one accumulation:
    out = local @ (0.7 Wo.T) + glob_pre @ (0.3 g_out_w.T @ Wo.T) + b.

All kernel inputs are staged host-side: weights are pre-transposed/scaled and
cast to fp16, activations cast to fp16, and the relative-embedding table is
expanded to the clip-extended REL_EXT form.
"""

import sys

sys.path.insert(0, "/opt/trn_rl_repo")

import math
from contextlib import ExitStack

import numpy as np

import concourse.bass as bass
import concourse.tile as tile
from concourse import bacc, mybir
from concourse.bass_utils import run_bass_kernel_spmd
from concourse.masks import make_identity

F16 = mybir.dt.float16
F32 = mybir.dt.float32

B, L, D = 256, 64, 1024
H, HD = 16, 64
HG, HDG = 8, 128
MAXREL = 32
NCORES = 8
BC = B // NCORES  # batches per core
NTOK = BC * L  # tokens per core
CH = 512  # tokens per chunk
NB = CH // L  # batches per chunk
NCHUNK = NTOK // CH
KT = D // 128  # contraction tiles
OT = D // 128  # output feature tiles

WNAMES = ["w_q", "w_gq", "w_k", "w_gk", "w_v", "w_gv", "w_o", "w_2"]


def _off(ap):
    return ap.offset


def _emit(nc, tc, ctx, io):
    pw = ctx.enter_context(tc.tile_pool(name="pw", bufs=3))
    px = ctx.enter_context(tc.tile_pool(name="px", bufs=2))
    pqk_l = ctx.enter_context(tc.tile_pool(name="pqk_l", bufs=2))
    pqk_g = ctx.enter_context(tc.tile_pool(name="pqk_g", bufs=1))
    pv = ctx.enter_context(tc.tile_pool(name="pv", bufs=1))
    plg = ctx.enter_context(tc.tile_pool(name="plg", bufs=1))
    pskew = ctx.enter_context(tc.tile_pool(name="pskew", bufs=1))
    pqksb = ctx.enter_context(tc.tile_pool(name="pqksb", bufs=1))
    psmall = ctx.enter_context(tc.tile_pool(name="psmall", bufs=4))
    pout = ctx.enter_context(tc.tile_pool(name="pout", bufs=3))
    pconst = ctx.enter_context(tc.tile_pool(name="pconst", bufs=1))
    pps_big = ctx.enter_context(tc.tile_pool(name="pps_big", bufs=2, space="PSUM"))
    pps_sc = ctx.enter_context(tc.tile_pool(name="pps_sc", bufs=3, space="PSUM"))
    pps_tr = ctx.enter_context(tc.tile_pool(name="pps_tr", bufs=2, space="PSUM"))
    pps_av = ctx.enter_context(tc.tile_pool(name="pps_av", bufs=1, space="PSUM"))
    pdram = ctx.enter_context(tc.tile_pool(name="pdram", bufs=2, space="DRAM"))

    ident = pconst.tile([128, 128], F16, tag="ident")
    make_identity(nc, ident[:])
    relt = pconst.tile([128, 127], F16, tag="relt")
    nc.sync.dma_start(relt[0:64, :], io["relt"][:])
    nc.sync.dma_start(relt[64:128, :], io["relt"][:])

    def load_w(name):
        wt = pw.tile([128, KT, D], F16, tag="w")
        nc.sync.dma_start(wt[:], io[name][:].rearrange("(ki p) o -> p ki o", p=128))
        return wt

    def load_xt(xname, tok0):
        xt = px.tile([128, KT, CH], F16, tag="x")
        for ki in range(KT):
            nc.sync.dma_start_transpose(
                xt[:, ki, :], io[xname][tok0 : tok0 + CH, ki * 128 : (ki + 1) * 128]
            )
        return xt

    def proj_t(xt, wt, dst):
        # dst[p, oi, t]: transposed projection output (features on partitions)
        for oi in range(OT):
            ps = pps_big.tile([128, CH], F32, tag="mm")
            for ki in range(KT):
                nc.tensor.matmul(
                    ps[:],
                    wt[:, ki, oi * 128 : (oi + 1) * 128],
                    xt[:, ki, :],
                    start=(ki == 0),
                    stop=(ki == KT - 1),
                )
            nc.vector.tensor_copy(out=dst[:, oi, :], in_=ps[:])

    def proj_n(xt, wt, dst):
        # dst[p, ts, o]: plain-layout projection output (tokens on partitions)
        for ts in range(NB // 2):
            for oh in range(2):
                ps = pps_big.tile([128, CH], F32, tag="mm")
                for ki in range(KT):
                    nc.tensor.matmul(
                        ps[:],
                        xt[:, ki, ts * 128 : (ts + 1) * 128],
                        wt[:, ki, oh * 512 : (oh + 1) * 512],
                        start=(ki == 0),
                        stop=(ki == KT - 1),
                    )
                nc.vector.tensor_copy(
                    out=dst[:, ts, oh * 512 : (oh + 1) * 512], in_=ps[:]
                )

    for c in range(NCHUNK):
        tok0 = c * CH

        # ---- input transposes (prefetch all three up front) ----
        xqt = load_xt("xq", tok0)
        xkt = load_xt("xk", tok0)
        xvt = load_xt("xv", tok0)

        # ---- projections ----
        wq = load_w("w_q")
        qt = pqk_l.tile([128, OT, CH], F16, tag="qt")
        proj_t(xqt, wq, qt)
        wgq = load_w("w_gq")
        qgt = pqk_g.tile([128, OT, CH], F16, tag="qgt")
        proj_t(xqt, wgq, qgt)

        wk = load_w("w_k")
        kt = pqk_l.tile([128, OT, CH], F16, tag="kt")
        proj_t(xkt, wk, kt)
        wgk = load_w("w_gk")
        kgt = pqk_g.tile([128, OT, CH], F16, tag="kgt")
        proj_t(xkt, wgk, kgt)

        wv = load_w("w_v")
        vt = pv.tile([128, NB // 2, D], F16, tag="v")
        proj_n(xvt, wv, vt)
        wgv = load_w("w_gv")
        vgt = pv.tile([128, NB // 2, D], F16, tag="vg")
        proj_n(xvt, wgv, vgt)

        lt = plg.tile([128, KT, CH], F16, tag="lt")
        gt = plg.tile([128, KT, CH], F16, tag="gt")

        # ---- local attention: scores + rel-position T matrices ----
        # two pair-tiles share one PSUM tile: [0:64]=qk0 [64:191]=T0
        # [192:256]=qk1 [256:383]=T1; drained with strided copies.
        tdr = pdram.tile([128, 64, 127], F16, tag="tdr")  # [bh, l, j]
        tap = tdr[:]
        qksb = pqksb.tile([128, 64, 64], F16, tag="qksb")
        for pt in range(0, 64, 2):
            bh0 = 2 * pt
            b = bh0 // H
            bcols = slice(b * L, (b + 1) * L)
            sc = pps_sc.tile([128, 384], F32, tag="sc")
            scv = sc[:].rearrange("p (u c) -> p u c", u=2)
            for u in range(2):
                j = ((bh0 + 2 * u) % H) // 2
                qa = qt[0:64, j, bcols]
                qb = qt[64:128, j, bcols]
                nc.tensor.matmul(
                    scv[0:64, u, 0:64], qa, kt[0:64, j, bcols], start=True, stop=True
                )
                nc.tensor.matmul(
                    scv[64:128, u, 0:64], qb, kt[64:128, j, bcols], start=True, stop=True
                )
                nc.tensor.matmul(
                    scv[0:64, u, 64:191], qa, relt[0:64, :], start=True, stop=True
                )
                nc.tensor.matmul(
                    scv[64:128, u, 64:191], qb, relt[64:128, :], start=True, stop=True
                )
            nc.vector.tensor_copy(out=qksb[:, pt : pt + 2, :], in_=scv[:, :, 0:64])
            tpart = psmall.tile([128, 2, 127], F16, tag="tpart")
            nc.scalar.copy(out=tpart[:], in_=scv[:, :, 64:191])
            # tdr layout [bh, l, j]: each store is one contiguous 32KB run
            for u in range(2):
                dst = bass.AP(
                    tap.tensor,
                    _off(tap) + (bh0 + 2 * u) * 64 * 127,
                    [[64 * 127, 2], [127, 64], [1, 127]],
                )
                nc.scalar.dma_start(dst, tpart[:, u, :])

        # skewed read: skew[p=pair*64+l, pt, r] = T[2*pt+pair, l, r-l+63]
        skew = pskew.tile([128, 64, 64], F16, tag="skew")
        for pair in range(2):
            src = bass.AP(
                tap.tensor,
                _off(tap) + 63 + pair * 64 * 127,
                [[126, 64], [2 * 64 * 127, 64], [1, 64]],
            )
            nc.scalar.dma_start(skew[pair * 64 : (pair + 1) * 64, :, :], src)

        # ---- global attention (independent of the skew roundtrip; emitted
        # here so PE has work while the T-store DMAs land) ----
        for g in range(8):
            b = g
            bcols = slice(b * L, (b + 1) * L)
            po = (b % 2) * 64
            sc = pps_sc.tile([128, 384], F32, tag="sc")
            for i in range(4):
                for pair in range(2):
                    hg = 2 * i + pair
                    nc.tensor.matmul(
                        sc[pair * 64 : (pair + 1) * 64, i * 64 : (i + 1) * 64],
                        qgt[:, hg, bcols],
                        kgt[:, hg, bcols],
                        start=True,
                        stop=True,
                    )
            gsb = psmall.tile([128, 4, 64], F32, tag="gsb")
            nc.vector.tensor_copy(out=gsb[:], in_=sc[:, 0:256])
            esb = psmall.tile([128, 4, 64], F16, tag="esb")
            nc.scalar.activation(esb[:], gsb[:], mybir.ActivationFunctionType.Exp)
            sums = psmall.tile([128, 4], F32, tag="sums")
            nc.vector.tensor_reduce(
                sums[:], esb[:], mybir.AxisListType.X, mybir.AluOpType.add
            )
            rcp = psmall.tile([128, 4], F32, tag="rcp")
            nc.vector.reciprocal(rcp[:], sums[:])
            attn = psmall.tile([128, 4, 64], F16, tag="attn")
            nc.vector.tensor_tensor(
                attn[:],
                esb[:],
                rcp[:, :, None].to_broadcast([128, 4, 64]),
                mybir.AluOpType.mult,
            )
            tr = pps_tr.tile([64, 512], F16, tag="tr")
            for i in range(4):
                nc.tensor.transpose(
                    tr[:, i * 128 : (i + 1) * 128], attn[:, i, :], ident[:]
                )
            attnt = psmall.tile([128, 4, 128], F16, tag="attnT")
            nc.scalar.copy(
                out=attnt[po : po + 64, :, :],
                in_=tr[:].rearrange("p (i c) -> p i c", i=4),
            )
            for half in range(2):
                avg = pps_av.tile([128, 256], F32, tag="av")
                for k in range(4):
                    hg = 4 * half + k
                    i, pair = hg // 2, hg % 2
                    vgsl = vgt[po : po + 64, b // 2, hg * 128 : (hg + 1) * 128]
                    nc.tensor.matmul(
                        avg[:, k * 64 : (k + 1) * 64],
                        vgsl,
                        attnt[po : po + 64, i, pair * 64 : (pair + 1) * 64],
                        start=True,
                        stop=True,
                    )
                nc.vector.tensor_copy(
                    out=gt[:, 4 * half : 4 * half + 4, bcols],
                    in_=avg[:].rearrange("p (k c) -> p k c", k=4),
                )

        # ---- local attention: softmax + attn @ v ----
        for g in range(16):
            b = g // 2
            j0 = 4 * (g % 2)
            bcols = slice(b * L, (b + 1) * L)
            po = (b % 2) * 64
            ssb = psmall.tile([128, 4, 64], F32, tag="ssb")
            nc.vector.tensor_tensor(
                ssb[:],
                qksb[:, 4 * g : 4 * g + 4, :],
                skew[:, 4 * g : 4 * g + 4, :],
                mybir.AluOpType.add,
            )
            esb = psmall.tile([128, 4, 64], F16, tag="esb")
            nc.scalar.activation(esb[:], ssb[:], mybir.ActivationFunctionType.Exp)
            sums = psmall.tile([128, 4], F32, tag="sums")
            nc.vector.tensor_reduce(
                sums[:], esb[:], mybir.AxisListType.X, mybir.AluOpType.add
            )
            rcp = psmall.tile([128, 4], F32, tag="rcp")
            nc.vector.reciprocal(rcp[:], sums[:])
            attn = psmall.tile([128, 4, 64], F16, tag="attn")
            nc.vector.tensor_tensor(
                attn[:],
                esb[:],
                rcp[:, :, None].to_broadcast([128, 4, 64]),
                mybir.AluOpType.mult,
            )
            tr = pps_tr.tile([64, 512], F16, tag="tr")
            for i in range(4):
                nc.tensor.transpose(
                    tr[:, i * 128 : (i + 1) * 128], attn[:, i, :], ident[:]
                )
            attnt = psmall.tile([128, 4, 128], F16, tag="attnT")
            nc.scalar.copy(
                out=attnt[po : po + 64, :, :],
                in_=tr[:].rearrange("p (i c) -> p i c", i=4),
            )
            av = pps_av.tile([128, 256], F32, tag="av")
            for i in range(4):
                for pair in range(2):
                    h = 2 * (j0 + i) + pair
                    vsl = vt[po : po + 64, b // 2, h * 64 : (h + 1) * 64]
                    nc.tensor.matmul(
                        av[pair * 64 : (pair + 1) * 64, i * 64 : (i + 1) * 64],
                        vsl,
                        attnt[po : po + 64, i, pair * 64 : (pair + 1) * 64],
                        start=True,
                        stop=True,
                    )
            nc.vector.tensor_copy(
                out=lt[:, j0 : j0 + 4, bcols],
                in_=av[:].rearrange("p (i c) -> p i c", i=4),
            )

        # ---- folded output projections ----
        wo = load_w("w_o")
        w2 = load_w("w_2")
        for ts in range(NB // 2):
            for oh in range(2):
                ps = pps_big.tile([128, CH], F32, tag="mm")
                for ki in range(KT):
                    nc.tensor.matmul(
                        ps[:],
                        lt[:, ki, ts * 128 : (ts + 1) * 128],
                        wo[:, ki, oh * 512 : (oh + 1) * 512],
                        start=(ki == 0),
                        stop=False,
                    )
                for ki in range(KT):
                    nc.tensor.matmul(
                        ps[:],
                        gt[:, ki, ts * 128 : (ts + 1) * 128],
                        w2[:, ki, oh * 512 : (oh + 1) * 512],
                        start=False,
                        stop=(ki == KT - 1),
                    )
                osb = pout.tile([128, 512], F32, tag="out")
                nc.vector.tensor_copy(out=osb[:], in_=ps[:])
                nc.scalar.dma_start(
                    io["out"][
                        tok0 + ts * 128 : tok0 + (ts + 1) * 128,
                        oh * 512 : (oh + 1) * 512,
                    ],
                    osb[:],
                )


_NC_CACHE = {}


def _get_module():
    if "nc" not in _NC_CACHE:
        nc = bacc.Bacc("TRN2", target_bir_lowering=False, debug=False)
        io = {}
        for nm in ("xq", "xk", "xv"):
            io[nm] = nc.dram_tensor(nm, [NTOK, D], F16, kind="ExternalInput").ap()
        for nm in WNAMES:
            io[nm] = nc.dram_tensor(nm, [D, D], F16, kind="ExternalInput").ap()
        io["relt"] = nc.dram_tensor("relt", [64, 127], F16, kind="ExternalInput").ap()
        io["out"] = nc.dram_tensor("out", [NTOK, D], F32, kind="ExternalOutput").ap()
        with tile.TileContext(nc) as tc, ExitStack() as ctx:
            _emit(nc, tc, ctx, io)
        nc.compile()
        _NC_CACHE["nc"] = nc
    return _NC_CACHE["nc"]


def _prepare_in_maps(inputs):
    f32 = lambda name: np.asarray(inputs[name], np.float32)

    wq_t = f32("Wq").T.astype(np.float16)
    wk_t = (f32("Wk").T / math.sqrt(HD)).astype(np.float16)
    wv_t = f32("Wv").T.astype(np.float16)
    g_in = f32("g_in_w")
    wgq_t = g_in[0:D].T.astype(np.float16)
    wgk_t = (g_in[D : 2 * D].T / math.sqrt(HDG)).astype(np.float16)
    wgv_t = g_in[2 * D : 3 * D].T.astype(np.float16)
    wo = f32("Wo")
    gow = f32("g_out_w")
    wo_t = (0.7 * wo.T).astype(np.float16)
    w2_t = (0.3 * (gow.T @ wo.T)).astype(np.float16)

    # biases are all zero in this problem's setup_inputs; the kernel folds
    # them away, so require that here rather than silently dropping them.
    for bname in ("bq", "bk", "bv", "bo", "g_in_b", "g_out_b"):
        assert not np.any(f32(bname)), f"nonzero bias {bname} not supported"

    rel_k = f32("rel_k")  # [2*MAXREL+1, HD]
    ext_ids = np.clip(np.arange(127) - 63, -MAXREL, MAXREL) + MAXREL
    relt = rel_k[ext_ids].T.astype(np.float16)  # [HD, 127]

    xq = np.asarray(inputs["query"], np.float32).astype(np.float16)
    xk = np.asarray(inputs["key"], np.float32).astype(np.float16)
    xv = np.asarray(inputs["value"], np.float32).astype(np.float16)

    shared = {
        "w_q": wq_t,
        "w_k": wk_t,
        "w_v": wv_t,
        "w_gq": wgq_t,
        "w_gk": wgk_t,
        "w_gv": wgv_t,
        "w_o": wo_t,
        "w_2": w2_t,
        "relt": np.ascontiguousarray(relt),
    }
    in_maps = []
    for ci in range(NCORES):
        sl = slice(ci * BC, (ci + 1) * BC)
        in_maps.append(
            {
                "xq": np.ascontiguousarray(xq[sl].reshape(NTOK, D)),
                "xk": np.ascontiguousarray(xk[sl].reshape(NTOK, D)),
                "xv": np.ascontiguousarray(xv[sl].reshape(NTOK, D)),
                **shared,
            }
        )
    return in_maps


def _run(inputs, **kwargs):
    nc = _get_module()
    in_maps = _prepare_in_maps(inputs)
    res = run_bass_kernel_spmd(nc, in_maps, core_ids=list(range(NCORES)), **kwargs)
    out = np.concatenate(
        [res.results[ci]["out"].reshape(BC, L, D) for ci in range(NCORES)], axis=0
    )
    return out, res


def kernel(**inputs) -> np.ndarray:
    out, _ = _run(inputs)
    return out


def kernel_profiled(**inputs):
    out, res = _run(inputs, trace=True)
    return out, res


# revision 18
# speedup vs baseline: 1.0528x; 1.0528x over previous
"""Trainium2 Bass kernel for EnhancedMultiHeadAttention.

Data-parallel over batch: B=256 split as 32 batches per core across 8 cores.
Each core runs an identical fused kernel over its 2048 tokens:

  - q/k/v + global q/k/v projections as fp16 matmuls (fp32 PSUM accumulate),
    with the 1/sqrt(hd) score scales folded into the k-projection weights.
  - local attention with relative-position bias: the Toeplitz term
    scores[l,r] += q[l] . rel_k[clip(r-l)] is computed as T = q @ REL_EXT.T
    followed by a skewed (diagonal) DMA read through a DRAM scratch buffer.
  - softmax without max-subtraction (scores are bounded), exp row-sums via
    the activation accumulator, per-partition scalar normalization.
  - the two output projections are folded into one accumulation:
    out = local @ (0.7 Wo.T) + glob_pre @ (0.3 g_out_w.T @ Wo.T) + b.

All kernel inputs are staged host-side: weights are pre-transposed/scaled and
cast to fp16, activations cast to fp16, and the relative-embedding table is
expanded to the clip-extended REL_EXT form.
"""

import sys

sys.path.insert(0, "/opt/trn_rl_repo")

import math
from contextlib import ExitStack

import numpy as np

import concourse.bass as bass
import concourse.tile as tile
from concourse import bacc, mybir
from concourse.bass_utils import run_bass_kernel_spmd
from concourse.masks import make_identity

F16 = mybir.dt.float16
F32 = mybir.dt.float32

B, L, D = 256, 64, 1024
H, HD = 16, 64
HG, HDG = 8, 128
MAXREL = 32
NCORES = 8
BC = B // NCORES  # batches per core
NTOK = BC * L  # tokens per core
CH = 512  # tokens per chunk
NB = CH // L  # batches per chunk
NCHUNK = NTOK // CH
KT = D // 128  # contraction tiles
OT = D // 128  # output feature tiles

WNAMES = ["w_q", "w_gq", "w_k", "w_gk", "w_v", "w_gv", "w_o", "w_2"]


def _off(ap):
    return ap.offset


def _emit(nc, tc, ctx, io):
    pw = ctx.enter_context(tc.tile_pool(name="pw", bufs=3))
    px = ctx.enter_context(tc.tile_pool(name="px", bufs=2))
    pqk_l = ctx.enter_context(tc.tile_pool(name="pqk_l", bufs=2))
    pqk_g = ctx.enter_context(tc.tile_pool(name="pqk_g", bufs=1))
    pv = ctx.enter_context(tc.tile_pool(name="pv", bufs=1))
    plg = ctx.enter_context(tc.tile_pool(name="plg", bufs=1))
    pskew = ctx.enter_context(tc.tile_pool(name="pskew", bufs=1))
    pqksb = ctx.enter_context(tc.tile_pool(name="pqksb", bufs=1))
    psmall = ctx.enter_context(tc.tile_pool(name="psmall", bufs=4))
    pout = ctx.enter_context(tc.tile_pool(name="pout", bufs=3))
    pconst = ctx.enter_context(tc.tile_pool(name="pconst", bufs=1))
    pps_big = ctx.enter_context(tc.tile_pool(name="pps_big", bufs=2, space="PSUM"))
    pps_sc = ctx.enter_context(tc.tile_pool(name="pps_sc", bufs=3, space="PSUM"))
    pps_tr = ctx.enter_context(tc.tile_pool(name="pps_tr", bufs=2, space="PSUM"))
    pps_av = ctx.enter_context(tc.tile_pool(name="pps_av", bufs=1, space="PSUM"))
    pdram = ctx.enter_context(tc.tile_pool(name="pdram", bufs=2, space="DRAM"))

    ident = pconst.tile([128, 128], F16, tag="ident")
    make_identity(nc, ident[:])
    relt = pconst.tile([128, 127], F16, tag="relt")
    nc.sync.dma_start(relt[0:64, :], io["relt"][:])
    nc.sync.dma_start(relt[64:128, :], io["relt"][:])

    def load_w(name):
        wt = pw.tile([128, KT, D], F16, tag="w")
        nc.sync.dma_start(wt[:], io[name][:].rearrange("(ki p) o -> p ki o", p=128))
        return wt

    def load_xt(xname, tok0):
        xt = px.tile([128, KT, CH], F16, tag="x")
        for ki in range(KT):
            nc.sync.dma_start_transpose(
                xt[:, ki, :], io[xname][tok0 : tok0 + CH, ki * 128 : (ki + 1) * 128]
            )
        return xt

    def proj_t(xt, wt, dst):
        # dst[p, oi, t]: transposed projection output (features on partitions)
        for oi in range(OT):
            ps = pps_big.tile([128, CH], F32, tag="mm")
            for ki in range(KT):
                nc.tensor.matmul(
                    ps[:],
                    wt[:, ki, oi * 128 : (oi + 1) * 128],
                    xt[:, ki, :],
                    start=(ki == 0),
                    stop=(ki == KT - 1),
                )
            nc.vector.tensor_copy(out=dst[:, oi, :], in_=ps[:])

    def proj_n(xt, wt, dst):
        # dst[p, ts, o]: plain-layout projection output (tokens on partitions)
        for ts in range(NB // 2):
            for oh in range(2):
                ps = pps_big.tile([128, CH], F32, tag="mm")
                for ki in range(KT):
                    nc.tensor.matmul(
                        ps[:],
                        xt[:, ki, ts * 128 : (ts + 1) * 128],
                        wt[:, ki, oh * 512 : (oh + 1) * 512],
                        start=(ki == 0),
                        stop=(ki == KT - 1),
                    )
                nc.vector.tensor_copy(
                    out=dst[:, ts, oh * 512 : (oh + 1) * 512], in_=ps[:]
                )

    for c in range(NCHUNK):
        tok0 = c * CH

        # ---- input transposes (prefetch all three up front) ----
        xqt = load_xt("xq", tok0)
        xkt = load_xt("xk", tok0)
        xvt = load_xt("xv", tok0)

        # ---- projections ----
        wq = load_w("w_q")
        qt = pqk_l.tile([128, OT, CH], F16, tag="qt")
        proj_t(xqt, wq, qt)
        wgq = load_w("w_gq")
        qgt = pqk_g.tile([128, OT, CH], F16, tag="qgt")
        proj_t(xqt, wgq, qgt)

        wk = load_w("w_k")
        kt = pqk_l.tile([128, OT, CH], F16, tag="kt")
        proj_t(xkt, wk, kt)
        wgk = load_w("w_gk")
        kgt = pqk_g.tile([128, OT, CH], F16, tag="kgt")
        proj_t(xkt, wgk, kgt)

        wv = load_w("w_v")
        vt = pv.tile([128, NB // 2, D], F16, tag="v")
        proj_n(xvt, wv, vt)
        wgv = load_w("w_gv")
        vgt = pv.tile([128, NB // 2, D], F16, tag="vg")
        proj_n(xvt, wgv, vgt)

        lt = plg.tile([128, KT, CH], F16, tag="lt")
        gt = plg.tile([128, KT, CH], F16, tag="gt")

        # ---- local attention: scores + rel-position T matrices ----
        # two pair-tiles share one PSUM tile: [0:64]=qk0 [64:191]=T0
        # [192:256]=qk1 [256:383]=T1; drained with strided copies.
        tdr = pdram.tile([128, 64, 127], F16, tag="tdr")  # [bh, l, j]
        tap = tdr[:]
        qksb = pqksb.tile([128, 64, 64], F16, tag="qksb")
        for pt in range(0, 64, 2):
            bh0 = 2 * pt
            b = bh0 // H
            bcols = slice(b * L, (b + 1) * L)
            sc = pps_sc.tile([128, 384], F32, tag="sc")
            scv = sc[:].rearrange("p (u c) -> p u c", u=2)
            for u in range(2):
                j = ((bh0 + 2 * u) % H) // 2
                qa = qt[0:64, j, bcols]
                qb = qt[64:128, j, bcols]
                nc.tensor.matmul(
                    scv[0:64, u, 0:64], qa, kt[0:64, j, bcols], start=True, stop=True
                )
                nc.tensor.matmul(
                    scv[64:128, u, 0:64], qb, kt[64:128, j, bcols], start=True, stop=True
                )
                nc.tensor.matmul(
                    scv[0:64, u, 64:191], qa, relt[0:64, :], start=True, stop=True
                )
                nc.tensor.matmul(
                    scv[64:128, u, 64:191], qb, relt[64:128, :], start=True, stop=True
                )
            nc.vector.tensor_copy(out=qksb[:, pt : pt + 2, :], in_=scv[:, :, 0:64])
            tpart = psmall.tile([128, 2, 127], F16, tag="tpart")
            nc.scalar.copy(out=tpart[:], in_=scv[:, :, 64:191])
            # tdr layout [bh, l, j]: each store is one contiguous 32KB run
            for u in range(2):
                dst = bass.AP(
                    tap.tensor,
                    _off(tap) + (bh0 + 2 * u) * 64 * 127,
                    [[64 * 127, 2], [127, 64], [1, 127]],
                )
                nc.scalar.dma_start(dst, tpart[:, u, :])

        # skewed read: skew[p=pair*64+l, pt, r] = T[2*pt+pair, l, r-l+63]
        skew = pskew.tile([128, 64, 64], F16, tag="skew")
        for pair in range(2):
            src = bass.AP(
                tap.tensor,
                _off(tap) + 63 + pair * 64 * 127,
                [[126, 64], [2 * 64 * 127, 64], [1, 64]],
            )
            nc.gpsimd.dma_start(skew[pair * 64 : (pair + 1) * 64, :, :], src)

        # ---- global attention (independent of the skew roundtrip; emitted
        # here so PE has work while the T-store DMAs land) ----
        for g in range(8):
            b = g
            bcols = slice(b * L, (b + 1) * L)
            po = (b % 2) * 64
            sc = pps_sc.tile([128, 384], F32, tag="sc")
            for i in range(4):
                for pair in range(2):
                    hg = 2 * i + pair
                    nc.tensor.matmul(
                        sc[pair * 64 : (pair + 1) * 64, i * 64 : (i + 1) * 64],
                        qgt[:, hg, bcols],
                        kgt[:, hg, bcols],
                        start=True,
                        stop=True,
                    )
            gsb = psmall.tile([128, 4, 64], F32, tag="gsb")
            nc.vector.tensor_copy(out=gsb[:], in_=sc[:, 0:256])
            esb = psmall.tile([128, 4, 64], F16, tag="esb")
            nc.scalar.activation(esb[:], gsb[:], mybir.ActivationFunctionType.Exp)
            sums = psmall.tile([128, 4], F32, tag="sums")
            nc.vector.tensor_reduce(
                sums[:], esb[:], mybir.AxisListType.X, mybir.AluOpType.add
            )
            rcp = psmall.tile([128, 4], F32, tag="rcp")
            nc.vector.reciprocal(rcp[:], sums[:])
            attn = psmall.tile([128, 4, 64], F16, tag="attn")
            nc.vector.tensor_tensor(
                attn[:],
                esb[:],
                rcp[:, :, None].to_broadcast([128, 4, 64]),
                mybir.AluOpType.mult,
            )
            tr = pps_tr.tile([64, 512], F16, tag="tr")
            for i in range(4):
                nc.tensor.transpose(
                    tr[:, i * 128 : (i + 1) * 128], attn[:, i, :], ident[:]
                )
            attnt = psmall.tile([128, 4, 128], F16, tag="attnT")
            nc.scalar.copy(
                out=attnt[po : po + 64, :, :],
                in_=tr[:].rearrange("p (i c) -> p i c", i=4),
            )
            for half in range(2):
                avg = pps_av.tile([128, 256], F32, tag="av")
                for k in range(4):
                    hg = 4 * half + k
                    i, pair = hg // 2, hg % 2
                    vgsl = vgt[po : po + 64, b // 2, hg * 128 : (hg + 1) * 128]
                    nc.tensor.matmul(
                        avg[:, k * 64 : (k + 1) * 64],
                        vgsl,
                        attnt[po : po + 64, i, pair * 64 : (pair + 1) * 64],
                        start=True,
                        stop=True,
                    )
                nc.vector.tensor_copy(
                    out=gt[:, 4 * half : 4 * half + 4, bcols],
                    in_=avg[:].rearrange("p (k c) -> p k c", k=4),
                )

        # ---- local attention: softmax + attn @ v ----
        for g in range(16):
            b = g // 2
            j0 = 4 * (g % 2)
            bcols = slice(b * L, (b + 1) * L)
            po = (b % 2) * 64
            ssb = psmall.tile([128, 4, 64], F32, tag="ssb")
            nc.vector.tensor_tensor(
                ssb[:],
                qksb[:, 4 * g : 4 * g + 4, :],
                skew[:, 4 * g : 4 * g + 4, :],
                mybir.AluOpType.add,
            )
            esb = psmall.tile([128, 4, 64], F16, tag="esb")
            nc.scalar.activation(esb[:], ssb[:], mybir.ActivationFunctionType.Exp)
            sums = psmall.tile([128, 4], F32, tag="sums")
            nc.vector.tensor_reduce(
                sums[:], esb[:], mybir.AxisListType.X, mybir.AluOpType.add
            )
            rcp = psmall.tile([128, 4], F32, tag="rcp")
            nc.vector.reciprocal(rcp[:], sums[:])
            attn = psmall.tile([128, 4, 64], F16, tag="attn")
            nc.vector.tensor_tensor(
                attn[:],
                esb[:],
                rcp[:, :, None].to_broadcast([128, 4, 64]),
                mybir.AluOpType.mult,
            )
            tr = pps_tr.tile([64, 512], F16, tag="tr")
            for i in range(4):
                nc.tensor.transpose(
                    tr[:, i * 128 : (i + 1) * 128], attn[:, i, :], ident[:]
                )
            attnt = psmall.tile([128, 4, 128], F16, tag="attnT")
            nc.scalar.copy(
                out=attnt[po : po + 64, :, :],
                in_=tr[:].rearrange("p (i c) -> p i c", i=4),
            )
            av = pps_av.tile([128, 256], F32, tag="av")
            for i in range(4):
                for pair in range(2):
                    h = 2 * (j0 + i) + pair
                    vsl = vt[po : po + 64, b // 2, h * 64 : (h + 1) * 64]
                    nc.tensor.matmul(
                        av[pair * 64 : (pair + 1) * 64, i * 64 : (i + 1) * 64],
                        vsl,
                        attnt[po : po + 64, i, pair * 64 : (pair + 1) * 64],
                        start=True,
                        stop=True,
                    )
            nc.vector.tensor_copy(
                out=lt[:, j0 : j0 + 4, bcols],
                in_=av[:].rearrange("p (i c) -> p i c", i=4),
            )

        # ---- folded output projections ----
        wo = load_w("w_o")
        w2 = load_w("w_2")
        for ts in range(NB // 2):
            for oh in range(2):
                ps = pps_big.tile([128, CH], F32, tag="mm")
                for ki in range(KT):
                    nc.tensor.matmul(
                        ps[:],
                        lt[:, ki, ts * 128 : (ts + 1) * 128],
                        wo[:, ki, oh * 512 : (oh + 1) * 512],
                        start=(ki == 0),
                        stop=False,
                    )
                for ki in range(KT):
                    nc.tensor.matmul(
                        ps[:],
                        gt[:, ki, ts * 128 : (ts + 1) * 128],
                        w2[:, ki, oh * 512 : (oh + 1) * 512],
                        start=False,
                        stop=(ki == KT - 1),
                    )
                osb = pout.tile([128, 512], F32, tag="out")
                nc.vector.tensor_copy(out=osb[:], in_=ps[:])
                nc.scalar.dma_start(
                    io["out"][
                        tok0 + ts * 128 : tok0 + (ts + 1) * 128,
                        oh * 512 : (oh + 1) * 512,
                    ],
                    osb[:],
                )


_NC_CACHE = {}


def _get_module():
    if "nc" not in _NC_CACHE:
        nc = bacc.Bacc("TRN2", target_bir_lowering=False, debug=False)
        io = {}
        for nm in ("xq", "xk", "xv"):
            io[nm] = nc.dram_tensor(nm, [NTOK, D], F16, kind="ExternalInput").ap()
        for nm in WNAMES:
            io[nm] = nc.dram_tensor(nm, [D, D], F16, kind="ExternalInput").ap()
        io["relt"] = nc.dram_tensor("relt", [64, 127], F16, kind="ExternalInput").ap()
        io["out"] = nc.dram_tensor("out", [NTOK, D], F32, kind="ExternalOutput").ap()
        with tile.TileContext(nc) as tc, ExitStack() as ctx:
            _emit(nc, tc, ctx, io)
        nc.compile()
        _NC_CACHE["nc"] = nc
    return _NC_CACHE["nc"]


def _prepare_in_maps(inputs):
    f32 = lambda name: np.asarray(inputs[name], np.float32)

    wq_t = f32("Wq").T.astype(np.float16)
    wk_t = (f32("Wk").T / math.sqrt(HD)).astype(np.float16)
    wv_t = f32("Wv").T.astype(np.float16)
    g_in = f32("g_in_w")
    wgq_t = g_in[0:D].T.astype(np.float16)
    wgk_t = (g_in[D : 2 * D].T / math.sqrt(HDG)).astype(np.float16)
    wgv_t = g_in[2 * D : 3 * D].T.astype(np.float16)
    wo = f32("Wo")
    gow = f32("g_out_w")
    wo_t = (0.7 * wo.T).astype(np.float16)
    w2_t = (0.3 * (gow.T @ wo.T)).astype(np.float16)

    # biases are all zero in this problem's setup_inputs; the kernel folds
    # them away, so require that here rather than silently dropping them.
    for bname in ("bq", "bk", "bv", "bo", "g_in_b", "g_out_b"):
        assert not np.any(f32(bname)), f"nonzero bias {bname} not supported"

    rel_k = f32("rel_k")  # [2*MAXREL+1, HD]
    ext_ids = np.clip(np.arange(127) - 63, -MAXREL, MAXREL) + MAXREL
    relt = rel_k[ext_ids].T.astype(np.float16)  # [HD, 127]

    xq = np.asarray(inputs["query"], np.float32).astype(np.float16)
    xk = np.asarray(inputs["key"], np.float32).astype(np.float16)
    xv = np.asarray(inputs["value"], np.float32).astype(np.float16)

    shared = {
        "w_q": wq_t,
        "w_k": wk_t,
        "w_v": wv_t,
        "w_gq": wgq_t,
        "w_gk": wgk_t,
        "w_gv": wgv_t,
        "w_o": wo_t,
        "w_2": w2_t,
        "relt": np.ascontiguousarray(relt),
    }
    in_maps = []
    for ci in range(NCORES):
        sl = slice(ci * BC, (ci + 1) * BC)
        in_maps.append(
            {
                "xq": np.ascontiguousarray(xq[sl].reshape(NTOK, D)),
                "xk": np.ascontiguousarray(xk[sl].reshape(NTOK, D)),
                "xv": np.ascontiguousarray(xv[sl].reshape(NTOK, D)),
                **shared,
            }
        )
    return in_maps


def _run(inputs, **kwargs):
    nc = _get_module()
    in_maps = _prepare_in_maps(inputs)
    res = run_bass_kernel_spmd(nc, in_maps, core_ids=list(range(NCORES)), **kwargs)
    out = np.concatenate(
        [res.results[ci]["out"].reshape(BC, L, D) for ci in range(NCORES)], axis=0
    )
    return out, res


def kernel(**inputs) -> np.ndarray:
    out, _ = _run(inputs)
    return out


def kernel_profiled(**inputs):
    out, res = _run(inputs, trace=True)
    return out, res


# revision 19
# speedup vs baseline: 1.2721x; 1.2083x over previous
"""Trainium2 Bass kernel for EnhancedMultiHeadAttention.

Data-parallel over batch: B=256 split as 32 batches per core across 8 cores.
Each core runs an identical fused kernel over its 2048 tokens:

  - q/k/v + global q/k/v projections as fp16 matmuls (fp32 PSUM accumulate),
    with the 1/sqrt(hd) score scales folded into the k-projection weights.
  - local attention with relative-position bias: the Toeplitz term
    scores[l,r] += q[l] . rel_k[clip(r-l)] is computed as T = q @ REL_EXT.T
    followed by a skewed (diagonal) DMA read through a DRAM scratch buffer.
  - softmax without max-subtraction (scores are bounded), exp row-sums via
    the activation accumulator, per-partition scalar normalization.
  - the two output projections are folded into one accumulation:
    out = local @ (0.7 Wo.T) + glob_pre @ (0.3 g_out_w.T @ Wo.T) + b.

All kernel inputs are staged host-side: weights are pre-transposed/scaled and
cast to fp16, activations cast to fp16, and the relative-embedding table is
expanded to the clip-extended REL_EXT form.
"""

import sys

sys.path.insert(0, "/opt/trn_rl_repo")

import math
from contextlib import ExitStack

import numpy as np

import concourse.bass as bass
import concourse.tile as tile
from concourse import bacc, mybir
from concourse.bass_utils import run_bass_kernel_spmd
from concourse.masks import make_identity

F16 = mybir.dt.float16
F32 = mybir.dt.float32

B, L, D = 256, 64, 1024
H, HD = 16, 64
HG, HDG = 8, 128
MAXREL = 32
NCORES = 8
BC = B // NCORES  # batches per core
NTOK = BC * L  # tokens per core
CH = 512  # tokens per chunk
NB = CH // L  # batches per chunk
NCHUNK = NTOK // CH
KT = D // 128  # contraction tiles
OT = D // 128  # output feature tiles

WNAMES = ["w_q", "w_gq", "w_k", "w_gk", "w_v", "w_gv", "w_o", "w_2"]


def _off(ap):
    return ap.offset


def _emit(nc, tc, ctx, io):
    pw = ctx.enter_context(tc.tile_pool(name="pw", bufs=3))
    px = ctx.enter_context(tc.tile_pool(name="px", bufs=2))
    pqk_l = ctx.enter_context(tc.tile_pool(name="pqk_l", bufs=2))
    pqk_g = ctx.enter_context(tc.tile_pool(name="pqk_g", bufs=1))
    pv = ctx.enter_context(tc.tile_pool(name="pv", bufs=1))
    plg = ctx.enter_context(tc.tile_pool(name="plg", bufs=1))
    pskew = ctx.enter_context(tc.tile_pool(name="pskew", bufs=1))
    pqksb = ctx.enter_context(tc.tile_pool(name="pqksb", bufs=1))
    psmall = ctx.enter_context(tc.tile_pool(name="psmall", bufs=4))
    pout = ctx.enter_context(tc.tile_pool(name="pout", bufs=3))
    pconst = ctx.enter_context(tc.tile_pool(name="pconst", bufs=1))
    pps_big = ctx.enter_context(tc.tile_pool(name="pps_big", bufs=2, space="PSUM"))
    pps_sc = ctx.enter_context(tc.tile_pool(name="pps_sc", bufs=3, space="PSUM"))
    pps_tr = ctx.enter_context(tc.tile_pool(name="pps_tr", bufs=2, space="PSUM"))
    pps_av = ctx.enter_context(tc.tile_pool(name="pps_av", bufs=1, space="PSUM"))
    pdram = ctx.enter_context(tc.tile_pool(name="pdram", bufs=2, space="DRAM"))

    ident = pconst.tile([128, 128], F16, tag="ident")
    make_identity(nc, ident[:])
    relt = pconst.tile([128, 127], F16, tag="relt")
    nc.sync.dma_start(relt[0:64, :], io["relt"][:])
    nc.sync.dma_start(relt[64:128, :], io["relt"][:])

    def load_w(name):
        wt = pw.tile([128, KT, D], F16, tag="w")
        nc.sync.dma_start(wt[:], io[name][:].rearrange("(ki p) o -> p ki o", p=128))
        return wt

    def load_xt(xname, tok0):
        xt = px.tile([128, KT, CH], F16, tag="x")
        nc.sync.dma_start(
            xt[:],
            io[xname][:, tok0 : tok0 + CH].rearrange("(ki p) t -> p ki t", p=128),
        )
        return xt

    def proj_t(xt, wt, dst):
        # dst[p, oi, t]: transposed projection output (features on partitions)
        for oi in range(OT):
            ps = pps_big.tile([128, CH], F32, tag="mm")
            for ki in range(KT):
                nc.tensor.matmul(
                    ps[:],
                    wt[:, ki, oi * 128 : (oi + 1) * 128],
                    xt[:, ki, :],
                    start=(ki == 0),
                    stop=(ki == KT - 1),
                )
            nc.vector.tensor_copy(out=dst[:, oi, :], in_=ps[:])

    def proj_n(xt, wt, dst):
        # dst[p, ts, o]: plain-layout projection output (tokens on partitions)
        for ts in range(NB // 2):
            osb = pout.tile([128, 1024], F32, tag="out")
            for oh in range(2):
                ps = pps_big.tile([128, CH], F32, tag="mm")
                for ki in range(KT):
                    nc.tensor.matmul(
                        ps[:],
                        xt[:, ki, ts * 128 : (ts + 1) * 128],
                        wt[:, ki, oh * 512 : (oh + 1) * 512],
                        start=(ki == 0),
                        stop=(ki == KT - 1),
                    )
                nc.vector.tensor_copy(
                    out=dst[:, ts, oh * 512 : (oh + 1) * 512], in_=ps[:]
                )

    for c in range(NCHUNK):
        tok0 = c * CH

        # ---- input transposes (prefetch all three up front) ----
        xqt = load_xt("xq", tok0)
        xkt = load_xt("xk", tok0)
        xvt = load_xt("xv", tok0)

        # ---- projections ----
        wq = load_w("w_q")
        qt = pqk_l.tile([128, OT, CH], F16, tag="qt")
        proj_t(xqt, wq, qt)
        wgq = load_w("w_gq")
        qgt = pqk_g.tile([128, OT, CH], F16, tag="qgt")
        proj_t(xqt, wgq, qgt)

        wk = load_w("w_k")
        kt = pqk_l.tile([128, OT, CH], F16, tag="kt")
        proj_t(xkt, wk, kt)
        wgk = load_w("w_gk")
        kgt = pqk_g.tile([128, OT, CH], F16, tag="kgt")
        proj_t(xkt, wgk, kgt)

        wv = load_w("w_v")
        vt = pv.tile([128, NB // 2, D], F16, tag="v")
        proj_n(xvt, wv, vt)
        wgv = load_w("w_gv")
        vgt = pv.tile([128, NB // 2, D], F16, tag="vg")
        proj_n(xvt, wgv, vgt)

        lt = plg.tile([128, KT, CH], F16, tag="lt")
        gt = plg.tile([128, KT, CH], F16, tag="gt")

        # ---- local attention: scores + rel-position T matrices ----
        # two pair-tiles share one PSUM tile: [0:64]=qk0 [64:191]=T0
        # [192:256]=qk1 [256:383]=T1; drained with strided copies.
        tdr = pdram.tile([2, 64, 64, 127], F16, tag="tdr")  # [pair, l, q, j]
        tap = tdr[:]
        qksb = pqksb.tile([128, 64, 64], F16, tag="qksb")
        for pt in range(0, 64, 2):
            bh0 = 2 * pt
            b = bh0 // H
            bcols = slice(b * L, (b + 1) * L)
            sc = pps_sc.tile([128, 384], F32, tag="sc")
            scv = sc[:].rearrange("p (u c) -> p u c", u=2)
            for u in range(2):
                j = ((bh0 + 2 * u) % H) // 2
                qa = qt[0:64, j, bcols]
                qb = qt[64:128, j, bcols]
                nc.tensor.matmul(
                    scv[0:64, u, 0:64], qa, kt[0:64, j, bcols], start=True, stop=True
                )
                nc.tensor.matmul(
                    scv[64:128, u, 0:64], qb, kt[64:128, j, bcols], start=True, stop=True
                )
                nc.tensor.matmul(
                    scv[0:64, u, 64:191], qa, relt[0:64, :], start=True, stop=True
                )
                nc.tensor.matmul(
                    scv[64:128, u, 64:191], qb, relt[64:128, :], start=True, stop=True
                )
            nc.vector.tensor_copy(out=qksb[:, pt : pt + 2, :], in_=scv[:, :, 0:64])
            tpart = psmall.tile([128, 2, 127], F16, tag="tpart")
            nc.scalar.copy(out=tpart[:], in_=scv[:, :, 64:191])
            # tdr[pair, l, q, j]: one 3-dim store covers both pair-tiles
            dst = bass.AP(
                tap.tensor,
                _off(tap) + pt * 127,
                [[64 * 64 * 127, 2], [64 * 127, 64], [1, 254]],
            )
            nc.scalar.dma_start(dst, tpart[:])

        # skewed read: skew[p=pair*64+l, pt, r] = T[pair, l, pt, r-l+63]
        skew = pskew.tile([128, 64, 64], F16, tag="skew")
        for pair in range(2):
            src = bass.AP(
                tap.tensor,
                _off(tap) + 63 + pair * 64 * 64 * 127,
                [[64 * 127 - 1, 64], [127, 64], [1, 64]],
            )
            nc.gpsimd.dma_start(skew[pair * 64 : (pair + 1) * 64, :, :], src)

        # ---- global attention (independent of the skew roundtrip; emitted
        # here so PE has work while the T-store DMAs land) ----
        for g in range(8):
            b = g
            bcols = slice(b * L, (b + 1) * L)
            po = (b % 2) * 64
            sc = pps_sc.tile([128, 384], F32, tag="sc")
            for i in range(4):
                for pair in range(2):
                    hg = 2 * i + pair
                    nc.tensor.matmul(
                        sc[pair * 64 : (pair + 1) * 64, i * 64 : (i + 1) * 64],
                        qgt[:, hg, bcols],
                        kgt[:, hg, bcols],
                        start=True,
                        stop=True,
                    )
            gsb = psmall.tile([128, 4, 64], F32, tag="gsb")
            nc.vector.tensor_copy(out=gsb[:], in_=sc[:, 0:256])
            esb = psmall.tile([128, 4, 64], F16, tag="esb")
            nc.scalar.activation(esb[:], gsb[:], mybir.ActivationFunctionType.Exp)
            sums = psmall.tile([128, 4], F32, tag="sums")
            nc.vector.tensor_reduce(
                sums[:], esb[:], mybir.AxisListType.X, mybir.AluOpType.add
            )
            rcp = psmall.tile([128, 4], F32, tag="rcp")
            nc.vector.reciprocal(rcp[:], sums[:])
            attn = psmall.tile([128, 4, 64], F16, tag="attn")
            nc.vector.tensor_tensor(
                attn[:],
                esb[:],
                rcp[:, :, None].to_broadcast([128, 4, 64]),
                mybir.AluOpType.mult,
            )
            tr = pps_tr.tile([64, 512], F16, tag="tr")
            for i in range(4):
                nc.tensor.transpose(
                    tr[:, i * 128 : (i + 1) * 128], attn[:, i, :], ident[:]
                )
            attnt = psmall.tile([128, 4, 128], F16, tag="attnT")
            nc.scalar.copy(
                out=attnt[po : po + 64, :, :],
                in_=tr[:].rearrange("p (i c) -> p i c", i=4),
            )
            for half in range(2):
                avg = pps_av.tile([128, 256], F32, tag="av")
                for k in range(4):
                    hg = 4 * half + k
                    i, pair = hg // 2, hg % 2
                    vgsl = vgt[po : po + 64, b // 2, hg * 128 : (hg + 1) * 128]
                    nc.tensor.matmul(
                        avg[:, k * 64 : (k + 1) * 64],
                        vgsl,
                        attnt[po : po + 64, i, pair * 64 : (pair + 1) * 64],
                        start=True,
                        stop=True,
                    )
                nc.vector.tensor_copy(
                    out=gt[:, 4 * half : 4 * half + 4, bcols],
                    in_=avg[:].rearrange("p (k c) -> p k c", k=4),
                )

        # ---- local attention: softmax + attn @ v ----
        for g in range(16):
            b = g // 2
            j0 = 4 * (g % 2)
            bcols = slice(b * L, (b + 1) * L)
            po = (b % 2) * 64
            ssb = psmall.tile([128, 4, 64], F32, tag="ssb")
            nc.vector.tensor_tensor(
                ssb[:],
                qksb[:, 4 * g : 4 * g + 4, :],
                skew[:, 4 * g : 4 * g + 4, :],
                mybir.AluOpType.add,
            )
            esb = psmall.tile([128, 4, 64], F16, tag="esb")
            nc.scalar.activation(esb[:], ssb[:], mybir.ActivationFunctionType.Exp)
            sums = psmall.tile([128, 4], F32, tag="sums")
            nc.vector.tensor_reduce(
                sums[:], esb[:], mybir.AxisListType.X, mybir.AluOpType.add
            )
            rcp = psmall.tile([128, 4], F32, tag="rcp")
            nc.vector.reciprocal(rcp[:], sums[:])
            attn = psmall.tile([128, 4, 64], F16, tag="attn")
            nc.vector.tensor_tensor(
                attn[:],
                esb[:],
                rcp[:, :, None].to_broadcast([128, 4, 64]),
                mybir.AluOpType.mult,
            )
            tr = pps_tr.tile([64, 512], F16, tag="tr")
            for i in range(4):
                nc.tensor.transpose(
                    tr[:, i * 128 : (i + 1) * 128], attn[:, i, :], ident[:]
                )
            attnt = psmall.tile([128, 4, 128], F16, tag="attnT")
            nc.scalar.copy(
                out=attnt[po : po + 64, :, :],
                in_=tr[:].rearrange("p (i c) -> p i c", i=4),
            )
            av = pps_av.tile([128, 256], F32, tag="av")
            for i in range(4):
                for pair in range(2):
                    h = 2 * (j0 + i) + pair
                    vsl = vt[po : po + 64, b // 2, h * 64 : (h + 1) * 64]
                    nc.tensor.matmul(
                        av[pair * 64 : (pair + 1) * 64, i * 64 : (i + 1) * 64],
                        vsl,
                        attnt[po : po + 64, i, pair * 64 : (pair + 1) * 64],
                        start=True,
                        stop=True,
                    )
            nc.vector.tensor_copy(
                out=lt[:, j0 : j0 + 4, bcols],
                in_=av[:].rearrange("p (i c) -> p i c", i=4),
            )

        # ---- folded output projections ----
        wo = load_w("w_o")
        w2 = load_w("w_2")
        for ts in range(NB // 2):
            osb = pout.tile([128, 1024], F32, tag="out")
            for oh in range(2):
                ps = pps_big.tile([128, CH], F32, tag="mm")
                for ki in range(KT):
                    nc.tensor.matmul(
                        ps[:],
                        lt[:, ki, ts * 128 : (ts + 1) * 128],
                        wo[:, ki, oh * 512 : (oh + 1) * 512],
                        start=(ki == 0),
                        stop=False,
                    )
                for ki in range(KT):
                    nc.tensor.matmul(
                        ps[:],
                        gt[:, ki, ts * 128 : (ts + 1) * 128],
                        w2[:, ki, oh * 512 : (oh + 1) * 512],
                        start=False,
                        stop=(ki == KT - 1),
                    )
                nc.vector.tensor_copy(out=osb[:, oh * 512 : (oh + 1) * 512], in_=ps[:])
            nc.scalar.dma_start(
                io["out"][tok0 + ts * 128 : tok0 + (ts + 1) * 128, :], osb[:]
            )


_NC_CACHE = {}


def _get_module():
    if "nc" not in _NC_CACHE:
        nc = bacc.Bacc("TRN2", target_bir_lowering=False, debug=False)
        io = {}
        for nm in ("xq", "xk", "xv"):
            io[nm] = nc.dram_tensor(nm, [D, NTOK], F16, kind="ExternalInput").ap()
        for nm in WNAMES:
            io[nm] = nc.dram_tensor(nm, [D, D], F16, kind="ExternalInput").ap()
        io["relt"] = nc.dram_tensor("relt", [64, 127], F16, kind="ExternalInput").ap()
        io["out"] = nc.dram_tensor("out", [NTOK, D], F32, kind="ExternalOutput").ap()
        with tile.TileContext(nc) as tc, ExitStack() as ctx:
            _emit(nc, tc, ctx, io)
        nc.compile()
        _NC_CACHE["nc"] = nc
    return _NC_CACHE["nc"]


def _prepare_in_maps(inputs):
    f32 = lambda name: np.asarray(inputs[name], np.float32)

    wq_t = f32("Wq").T.astype(np.float16)
    wk_t = (f32("Wk").T / math.sqrt(HD)).astype(np.float16)
    wv_t = f32("Wv").T.astype(np.float16)
    g_in = f32("g_in_w")
    wgq_t = g_in[0:D].T.astype(np.float16)
    wgk_t = (g_in[D : 2 * D].T / math.sqrt(HDG)).astype(np.float16)
    wgv_t = g_in[2 * D : 3 * D].T.astype(np.float16)
    wo = f32("Wo")
    gow = f32("g_out_w")
    wo_t = (0.7 * wo.T).astype(np.float16)
    w2_t = (0.3 * (gow.T @ wo.T)).astype(np.float16)

    # biases are all zero in this problem's setup_inputs; the kernel folds
    # them away, so require that here rather than silently dropping them.
    for bname in ("bq", "bk", "bv", "bo", "g_in_b", "g_out_b"):
        assert not np.any(f32(bname)), f"nonzero bias {bname} not supported"

    rel_k = f32("rel_k")  # [2*MAXREL+1, HD]
    ext_ids = np.clip(np.arange(127) - 63, -MAXREL, MAXREL) + MAXREL
    relt = rel_k[ext_ids].T.astype(np.float16)  # [HD, 127]

    xq = np.asarray(inputs["query"], np.float32).astype(np.float16)
    xk = np.asarray(inputs["key"], np.float32).astype(np.float16)
    xv = np.asarray(inputs["value"], np.float32).astype(np.float16)

    shared = {
        "w_q": wq_t,
        "w_k": wk_t,
        "w_v": wv_t,
        "w_gq": wgq_t,
        "w_gk": wgk_t,
        "w_gv": wgv_t,
        "w_o": wo_t,
        "w_2": w2_t,
        "relt": np.ascontiguousarray(relt),
    }
    in_maps = []
    for ci in range(NCORES):
        sl = slice(ci * BC, (ci + 1) * BC)
        in_maps.append(
            {
                "xq": np.ascontiguousarray(xq[sl].reshape(NTOK, D).T),
                "xk": np.ascontiguousarray(xk[sl].reshape(NTOK, D).T),
                "xv": np.ascontiguousarray(xv[sl].reshape(NTOK, D).T),
                **shared,
            }
        )
    return in_maps


def _run(inputs, **kwargs):
    nc = _get_module()
    in_maps = _prepare_in_maps(inputs)
    res = run_bass_kernel_spmd(nc, in_maps, core_ids=list(range(NCORES)), **kwargs)
    out = np.concatenate(
        [res.results[ci]["out"].reshape(BC, L, D) for ci in range(NCORES)], axis=0
    )
    return out, res


def kernel(**inputs) -> np.ndarray:
    out, _ = _run(inputs)
    return out


def kernel_profiled(**inputs):
    out, res = _run(inputs, trace=True)
    return out, res
